# revision 7
# baseline (speedup 1.0000x reference)
"""DensityModulatedSelfAttention TRN2 Bass kernel.

Sharding: query-split across 8 cores — core c handles batch b = c//2 and
query rows q0 = (c%2)*512 .. +512.  Each core computes full K/V for its
batch (duplicated across the pair), its 512 query rows of attention for
all 12 heads, the output projection, residual + LayerNorm — no
collectives.

Key layout trick: projections are computed TRANSPOSED (qT/kT with the
head-dim on partitions) by feeding x^T so the scores matmuls need no
on-chip transposes. Scores are computed twice, once per layout:
  pass A [nq, m]: softmax along free dim -> attn output to HBM
  pass B [m, nq]: exp'd tiles feed the PV matmul directly as lhsT with a
    fused ones-column producing the row-sums; out^T lands head-major so
    the out-projection consumes it with no transpose either.
The density modulation (1+d)/sqrt(HD) is folded into x's query rows on
the host (exact algebraic refactor), so scores arrive pre-modulated.

All matmuls run as float32r (TF32-like, 1 col/cycle, rel err ~1.6e-4).
"""

import numpy as np

import concourse.bacc as bacc
import concourse.mybir as mybir
import concourse.tile as tile
from concourse.bass import ts
from concourse.bass_utils import run_bass_kernel_spmd

B, N, D, H, HD = 4, 1024, 768, 12, 64
NQ = 512            # query rows per core
NCORES = 8
EPS = 1e-5
F32 = mybir.dt.float32
F32R = mybir.dt.float32r
AF = mybir.ActivationFunctionType
OP = mybir.AluOpType

# aux row layout (all on partition 0): [bq | bk | bv | bo | s_row | ln_g | ln_b]
BQ_OFF, BK_OFF, BV_OFF, BO_OFF = 0, D, 2 * D, 3 * D
S_OFF = 4 * D
LNG_OFF = 4 * D + NQ
LNB_OFF = 5 * D + NQ
ONES_OFF = 6 * D + NQ
AUXF = 6 * D + NQ + N

# blob_a columns: [xT (1024) | WkT (768) | WvT (768)]
BA_XT, BA_WK, BA_WV = 0, N, N + D
BAF = N + 2 * D
# blob_b columns: [xqsT (512) | WqT (768)]
BB_XQS, BB_WQ = 0, NQ
BBF = NQ + D

DC = D // 128    # 6 contraction chunks
OC = D // 128    # 6 output chunks of qT/kT
MC = N // 128    # 8 key chunks
QC = NQ // 128   # 4 query-row chunks


def build_kernel(n_iters: int = 1):
    nc = bacc.Bacc()
    blob_a = nc.dram_tensor("blob_a", [D, BAF], F32R, kind="ExternalInput")
    blob_b = nc.dram_tensor("blob_b", [D, BBF], F32R, kind="ExternalInput")
    wo_d = nc.dram_tensor("woT", [D, D], F32R, kind="ExternalInput")
    aux_d = nc.dram_tensor("aux", [1, AUXF], F32R, kind="ExternalInput")
    onesc_d = nc.dram_tensor("onesc", [128, MC, H, 1], F32R, kind="ExternalInput")
    xr_d = nc.dram_tensor("xr", [NQ, D], F32, kind="ExternalInput")
    attn_o = nc.dram_tensor("attn_part", [H, NQ, N], F32, kind="ExternalOutput")
    out_o = nc.dram_tensor("out_part", [NQ, D], F32, kind="ExternalOutput")

    with tile.TileContext(nc) as tc:

        def body(_iv=None):
            with tc.tile_pool(name="persist", bufs=1) as pp:
                kT = pp.tile([128, OC, N], F32R)       # kT[o%128, o//128, m]
                v_aug = pp.tile([128, MC, H, HD + 1], F32R)  # [m%128, mc, h, hd|1]
                qT = pp.tile([128, OC, NQ], F32R)      # qT_scaled
                aoT = pp.tile([128, OC, NQ], F32R)     # normalized out^T, pair-stacked
                aux = pp.tile([1, AUXF], F32R)
                nc.sync.dma_start(aux, aux_d[:, :])
                ones_row = aux[0:1, ONES_OFF : ONES_OFF + N]
                nc.sync.dma_start(v_aug[:, :, :, HD : HD + 1], onesc_d[:, :, :, :])

                # ---------------- phase 1a: kT and v ----------------
                with (
                    tc.tile_pool(name="p1a", bufs=1) as p1a,
                    tc.tile_pool(name="ps1", bufs=2, space="PSUM") as ps1,
                ):
                    ba = p1a.tile([128, DC, BAF], F32R)
                    nc.sync.dma_start(ba, blob_a.rearrange("(c p) f -> p c f", p=128))
                    for j in range(OC):  # kT output chunks
                        pk = ps1.tile([128, N], F32, tag="pk")
                        for i in range(DC):
                            for hf in range(2):
                                nc.tensor.matmul(
                                    pk[:, ts(hf, 512)],
                                    ba[:, i, BA_WK + j * 128 : BA_WK + (j + 1) * 128],
                                    ba[:, i, BA_XT + hf * 512 : BA_XT + hf * 512 + 512],
                                    start=(i == 0),
                                    stop=False,
                                )
                        for hf in range(2):  # + bk ⊗ 1
                            nc.tensor.matmul(
                                pk[:, ts(hf, 512)],
                                aux[0:1, BK_OFF + j * 128 : BK_OFF + (j + 1) * 128],
                                ones_row[0:1, ts(hf, 512)],
                                start=False,
                                stop=True,
                            )
                        nc.vector.tensor_copy(kT[:, j, :], pk)
                    for mc in range(MC):  # v output chunks
                        # halves at 512-aligned offsets: one PSUM bank per matmul
                        pv_ = ps1.tile([128, 1024], F32, tag="pv")
                        for i in range(DC):
                            for hf in range(2):
                                nc.tensor.matmul(
                                    pv_[:, hf * 512 : hf * 512 + 384],
                                    ba[:, i, BA_XT + mc * 128 : BA_XT + (mc + 1) * 128],
                                    ba[:, i, BA_WV + hf * 384 : BA_WV + hf * 384 + 384],
                                    start=(i == 0),
                                    stop=False,
                                )
                        for hf in range(2):  # + 1 ⊗ bv
                            nc.tensor.matmul(
                                pv_[:, hf * 512 : hf * 512 + 384],
                                ones_row[0:1, 0:128],
                                aux[0:1, BV_OFF + hf * 384 : BV_OFF + hf * 384 + 384],
                                start=False,
                                stop=True,
                            )
                        for hf in range(2):
                            nc.vector.tensor_copy(
                                v_aug[:, mc, hf * 6 : hf * 6 + 6, 0:HD],
                                pv_[:, hf * 512 : hf * 512 + 384].rearrange(
                                    "p (h e) -> p h e", e=HD
                                ),
                            )

                # ---------------- phase 1b: qT (pre-scaled) ----------------
                with (
                    tc.tile_pool(name="p1b", bufs=1) as p1b,
                    tc.tile_pool(name="ps2", bufs=2, space="PSUM") as ps2,
                ):
                    bb = p1b.tile([128, DC, BBF], F32R)
                    nc.sync.dma_start(bb, blob_b.rearrange("(c p) f -> p c f", p=128))
                    for j in range(OC):
                        pq = ps2.tile([128, NQ], F32, tag="pq")
                        for i in range(DC):
                            nc.tensor.matmul(
                                pq,
                                bb[:, i, BB_WQ + j * 128 : BB_WQ + (j + 1) * 128],
                                bb[:, i, BB_XQS : BB_XQS + NQ],
                                start=(i == 0),
                                stop=False,
                            )
                        nc.tensor.matmul(  # + bq ⊗ s
                            pq,
                            aux[0:1, BQ_OFF + j * 128 : BQ_OFF + (j + 1) * 128],
                            aux[0:1, S_OFF : S_OFF + NQ],
                            start=False,
                            stop=True,
                        )
                        nc.vector.tensor_copy(qT[:, j, :], pq)

                # ---------------- phase 2a: attn output [nq, m] ----------------
                with (
                    tc.tile_pool(name="p2a", bufs=1) as p2a,
                    tc.tile_pool(name="psA", bufs=3, space="PSUM") as psA,
                ):
                    for h in range(H):
                        j, p0 = h // 2, (h % 2) * 64
                        for nqc in range(QC):
                            sa = psA.tile([128, N], F32, tag="sa")
                            for hf in range(2):
                                nc.tensor.matmul(
                                    sa[:, ts(hf, 512)],
                                    qT[p0 : p0 + 64, j, ts(nqc, 128)],
                                    kT[p0 : p0 + 64, j, ts(hf, 512)],
                                    start=True,
                                    stop=True,
                                )
                            at = p2a.tile([128, N], F32, tag="at", bufs=3)
                            rs = p2a.tile([128, 1], F32, tag="rs", bufs=4)
                            nc.scalar.activation(at, sa, AF.Exp, accum_out=rs)
                            rr = p2a.tile([128, 1], F32, tag="rr", bufs=4)
                            nc.vector.reciprocal(rr, rs)
                            nc.vector.tensor_scalar_mul(at, at, rr)
                            nc.sync.dma_start(attn_o[h, ts(nqc, 128), :], at)

                # ---------------- phase 2b: PV -> aoT ----------------
                with (
                    tc.tile_pool(name="p2b", bufs=1) as p2b,
                    tc.tile_pool(name="psB", bufs=1, space="PSUM") as psB,
                ):
                    wo = pp.tile([128, OC, D], F32R)
                    nc.sync.dma_start(wo, wo_d.rearrange("(c p) f -> p c f", p=128))
                    for h in range(H):
                        j, p0 = h // 2, (h % 2) * 64
                        pvp = psB.tile([65, 512], F32, tag="pv", bufs=2)
                        for half in range(4):
                            sb_ = psB.tile([128, 1024], F32, tag="sb", bufs=2)
                            for k in range(2):
                                mc = half * 2 + k
                                nc.tensor.matmul(
                                    sb_[:, ts(k, 512)],
                                    kT[p0 : p0 + 64, j, ts(mc, 128)],
                                    qT[p0 : p0 + 64, j, :],
                                    start=True,
                                    stop=True,
                                )
                            ex = p2b.tile([128, 1024], F32R, tag="ex", bufs=3)
                            nc.scalar.activation(ex, sb_, AF.Exp)
                            for k in range(2):
                                mc = half * 2 + k
                                nc.tensor.matmul(
                                    pvp,
                                    v_aug[:, mc, h, :],
                                    ex[:, ts(k, 512)],
                                    start=(mc == 0),
                                    stop=(mc == MC - 1),
                                )
                        rt = p2b.tile([1, 512], F32R, tag="rt", bufs=2)
                        with nc.allow_low_precision(
                            reason="f32r rowsum recip feeds f32r bcast matmul"
                        ):
                            nc.vector.reciprocal(rt, pvp[64:65, :])
                        rbp = psB.tile([64, 512], F32, tag="rb", bufs=2)
                        nc.tensor.matmul(
                            rbp, ones_row[0:1, 0:64], rt, start=True, stop=True
                        )
                        rb_sb = p2b.tile([64, 512], F32, tag="rbs", bufs=2)
                        nc.vector.tensor_copy(rb_sb, rbp)
                        nc.vector.tensor_mul(
                            aoT[p0 : p0 + 64, j, :], pvp[0:64, :], rb_sb
                        )

                # ---------------- phase 3: out-proj + residual + LN ----------------
                with (
                    tc.tile_pool(name="p3", bufs=1) as p3,
                    tc.tile_pool(name="psC", bufs=1, space="PSUM") as psC,
                ):
                    xr = p3.tile([128, QC, D], F32)
                    nc.sync.dma_start(xr, xr_d.rearrange("(c p) f -> p c f", p=128))
                    g_sb = p3.tile([128, D], F32, tag="gb", bufs=2)
                    b_sb = p3.tile([128, D], F32, tag="gb", bufs=2)
                    for dst, off in ((g_sb, LNG_OFF), (b_sb, LNB_OFF)):
                        gp = psC.tile([128, 1024], F32, tag="g", bufs=2)
                        for hf in range(2):
                            nc.tensor.matmul(
                                gp[:, hf * 512 : hf * 512 + 384],
                                ones_row[0:1, 0:128],
                                aux[0:1, off + hf * 384 : off + hf * 384 + 384],
                                start=True,
                                stop=True,
                            )
                        nc.vector.tensor_copy(
                            dst.rearrange("p (g f) -> p g f", f=384),
                            gp.rearrange("p (g f) -> p g f", f=512)[:, :, 0:384],
                        )
                    for nqc in range(QC):
                        op_ = psC.tile([128, 1024], F32, tag="op", bufs=2)
                        for hf in range(2):
                            for j in range(OC):
                                nc.tensor.matmul(
                                    op_[:, hf * 512 : hf * 512 + 384],
                                    aoT[:, j, ts(nqc, 128)],
                                    wo[:, j, hf * 384 : hf * 384 + 384],
                                    start=(j == 0),
                                    stop=False,
                                )
                            nc.tensor.matmul(  # + 1 ⊗ bo
                                op_[:, hf * 512 : hf * 512 + 384],
                                ones_row[0:1, 0:128],
                                aux[0:1, BO_OFF + hf * 384 : BO_OFF + hf * 384 + 384],
                                start=False,
                                stop=True,
                            )
                        z = p3.tile([128, D], F32, tag="z", bufs=2)
                        nc.vector.tensor_add(
                            z.rearrange("p (g f) -> p g f", f=384),
                            op_.rearrange("p (g f) -> p g f", f=512)[:, :, 0:384],
                            xr[:, nqc, :].rearrange("p (g f) -> p g f", f=384),
                        )
                        bns = p3.tile([128, 12], F32, tag="bns", bufs=2)
                        nc.vector.bn_stats(bns[:, 0:6], z[:, 0:384])
                        nc.vector.bn_stats(bns[:, 6:12], z[:, 384:768])
                        mv = p3.tile([128, 2], F32, tag="mv", bufs=2)
                        nc.vector.bn_aggr(mv, bns)
                        ve = p3.tile([128, 1], F32, tag="ve", bufs=2)
                        nc.vector.tensor_scalar_add(ve, mv[:, 1:2], EPS)
                        sd = p3.tile([128, 1], F32, tag="sd", bufs=2)
                        nc.scalar.activation(sd, ve, AF.Sqrt)
                        r0 = p3.tile([128, 1], F32, tag="r0", bufs=2)
                        nc.vector.reciprocal(r0, sd)
                        # one Newton step: r1 = r0 * (1.5 - 0.5 * ve * r0^2)
                        t = p3.tile([128, 1], F32, tag="t", bufs=2)
                        nc.vector.tensor_mul(t, r0, r0)
                        nc.vector.tensor_mul(t, t, ve)
                        nc.vector.tensor_scalar(t, t, -0.5, 1.5, OP.mult, OP.add)
                        r1 = p3.tile([128, 1], F32, tag="r1", bufs=2)
                        nc.vector.tensor_mul(r1, r0, t)
                        zn = p3.tile([128, D], F32, tag="zn", bufs=2)
                        nc.vector.tensor_scalar(
                            zn, z, mv[:, 0:1], r1, OP.subtract, OP.mult
                        )
                        nc.vector.tensor_mul(zn, zn, g_sb)
                        nc.vector.tensor_add(zn, zn, b_sb)
                        nc.sync.dma_start(out_o[ts(nqc, 128), :], zn)

        if n_iters == 1:
            body()
        else:
            with tc.For_i(0, n_iters, 1) as iv:
                body(iv)

    nc.compile()
    return nc


def _prep_core_inputs(inputs, c):
    b, q0 = c // 2, (c % 2) * NQ
    x_b = np.asarray(inputs["x"][b], np.float32)
    d_b = np.asarray(inputs["density_weights"][b], np.float32).reshape(N)
    s = (1.0 + d_b) / np.sqrt(HD)
    f32 = lambda a: np.ascontiguousarray(np.asarray(a, np.float32))
    xT = f32(x_b.T)
    xqsT = f32((x_b[q0 : q0 + NQ] * s[q0 : q0 + NQ, None]).T)
    blob_a = f32(np.concatenate([xT, f32(inputs["Wk"]).T, f32(inputs["Wv"]).T], axis=1))
    blob_b = f32(np.concatenate([xqsT, f32(inputs["Wq"]).T], axis=1))
    aux = f32(
        np.concatenate(
            [
                np.asarray(inputs["bq"], np.float32),
                np.asarray(inputs["bk"], np.float32),
                np.asarray(inputs["bv"], np.float32),
                np.asarray(inputs["bo"], np.float32),
                s[q0 : q0 + NQ],
                np.asarray(inputs["ln_g"], np.float32),
                np.asarray(inputs["ln_b"], np.float32),
                np.ones(N, np.float32),
            ]
        )[None, :]
    )
    return {
        "blob_a": blob_a,
        "blob_b": blob_b,
        "woT": f32(f32(inputs["Wo"]).T),
        "aux": aux,
        "onesc": np.ones((128, MC, H, 1), np.float32),
        "xr": f32(x_b[q0 : q0 + NQ]),
    }


_NC = None


def kernel(**inputs):
    global _NC
    if _NC is None:
        _NC = build_kernel()
    in_maps = [_prep_core_inputs(inputs, c) for c in range(NCORES)]
    res = run_bass_kernel_spmd(_NC, in_maps, list(range(NCORES)))
    out = np.empty((B, N, D), np.float32)
    attn = np.empty((B, H, N, N), np.float32)
    for c in range(NCORES):
        b, q0 = c // 2, (c % 2) * NQ
        attn[b, :, q0 : q0 + NQ, :] = res.results[c]["attn_part"]
        out[b, q0 : q0 + NQ, :] = res.results[c]["out_part"]
    return out, attn
